# revision 3
# baseline (speedup 1.0000x reference)
"""GNN segment-product kernel v3 for 8 Trainium2 NeuronCores.

Computation:
    h = x @ W                                  [N, 64]
    prod[d] = product of h[src[e]] over incoming edges of d (1 if none)
    neigh = where(deg > 0, prod, 0)
    out = neigh @ V.T                          [N, 256]

v3 design (vs v2's GPSIMD ap_gather): the per-edge gather runs as SWDGE
dma_gather (dynamic-descriptor DMA, transpose=False) straight from a DRAM
fp32 h-table.  Table rows are 256B (64 fp32 features) at 1024B stride, 4
chunk sub-tables interleaved so int16 indices stay < 32767.  Gathered rows
land dst-major ([dst partition, slot column, feature]); fold trees on DVE
multiply the K slots per dst, 3 muls combine the 4 chunk partials, a PE
identity-matmul transposes each 128-dst group and a second matmul applies
V^T.  Gathers are split into <=896-idx instructions (the SWDGE ring holds
64 descriptors/engine) spread over 4 SWDGE queues for pipelining (transpose
mode would corrupt across queues -- the XBAR is shared; plain gathers are
safe).  A greedy host-side balancer spreads each dst's in-edges over the 4
chunks so K ~= ceil(deg/4).  A dummy gather at t=0 prefetches the mlp
GPSIMD library.
"""

import math
import numpy as np
from contextlib import ExitStack

import concourse.bass as bass
import concourse.bacc as bacc
import concourse.mybir as mybir
import concourse.tile as tile
from concourse import bass_utils

import os

P = 128
NCORES = 8
NCH = 4           # chunk sub-tables (int16 index limit: 8*W < 32768)
G = 128           # dsts per group (one PE output tile)
CAP_IDX = 3584    # max gather indices per (batch, chunk) staging tile
SUB = 896         # idxs per dma_gather instr (ring: 58<=64 descs)
NQ = 4           # SWDGE queues (parallel desc-gen pairs + rings)
MAXB = 16         # max groups per batch
ELEM = 64         # payload elems per table row (128B bf16)
ROWBF = 128       # bf16 elems between consecutive rows (256B stride)
CAPC = CAP_IDX // 128  # staging columns per (batch, chunk)






def _dma_gather_raw(nc, out_ap, in_ap, idxs_ap, num_idxs, elem_size,
                    elem_step, queue_num):
    """nc.gpsimd.dma_gather minus the elem_size_bytes%256 assert, which only
    applies to transpose mode (xbar desc granularity); the non-transpose
    ucode path packetizes any elem size."""
    import concourse.mybir as mybir
    from concourse._compat import exact_div
    gp = nc.gpsimd
    assert idxs_ap.dtype == mybir.dt.int16
    assert in_ap.dtype == out_ap.dtype
    stride_bytes = elem_step * mybir.dt.size(in_ap.dtype)
    stride_bytes_256 = exact_div(stride_bytes, 256)
    _in_ap = gp.lower_ap_dma(in_ap, for_custom_bir_dma=True)
    _idxs_ap = gp.lower_ap(idxs_ap)
    _out_ap = gp.lower_ap(out_ap)
    return gp.add_instruction(
        mybir.InstDMAGatherAnt(
            name=nc.get_next_instruction_name(),
            ins=[*_in_ap, _idxs_ap, gp.lower_val_access(gp.to_reg(num_idxs))],
            outs=[_out_ap],
            transpose=False,
            num_idxs=num_idxs,
            elem_size=elem_size,
            stride_bytes_256=stride_bytes_256,
            gen_mode=0,
            single_packet=True,
            queue_num=queue_num,
            sbuf_tokens_per_rank=0,
            sbuf_free_dim_per_rank=0,
            sbuf_free_dim_pad_per_rank=0,
            sbuf_byte_offset=0,
        )
    )

def _host_prep(x, W, V, src, dst):
    N, F = x.shape
    R = W.shape[1]
    H = V.shape[0]
    npc = N // NCORES

    deg = np.bincount(dst, minlength=N)

    # ---- table-side ownership: node n's h computed by core n % 8 ----
    owner_h = np.arange(N) % NCORES

    sorder = np.argsort(src, kind="stable")
    dst_by_src = dst[sorder]
    odeg = np.bincount(src, minlength=N)
    sstart = np.zeros(N + 1, np.int64)
    np.cumsum(odeg, out=sstart[1:])

    # ---- greedy chunk assignment: balance each dst's edges over 4 chunks,
    #      with per-(chunk, owner core) node-count caps ----
    cnt = np.zeros((N, NCH), np.int16)
    qa = np.zeros(N, np.int8)
    tgt = (deg / NCH).astype(np.float32)
    wqc = np.zeros((NCH, NCORES), np.int32)
    capw = npc // NCH + 48
    order = np.argsort(-odeg, kind="stable")
    for n in order:
        ds = dst_by_src[sstart[n]:sstart[n + 1]]
        c = owner_h[n]
        room = wqc[:, c] < capw
        if len(ds) == 0:
            q = int(np.argmin(np.where(room, wqc[:, c], 1 << 30)))
        else:
            co = cnt[ds].astype(np.float32) - tgt[ds][:, None]
            cost = np.maximum(co + 1.0, 0.0).sum(axis=0)
            q = int(np.argmin(np.where(room, cost, np.inf)))
        qa[n] = q
        wqc[q, c] += 1
        if len(ds):
            cnt[ds, q] += 1
    for _pass in range(2):  # refinement
        for n in order:
            ds = dst_by_src[sstart[n]:sstart[n + 1]]
            if len(ds) == 0:
                continue
            c = owner_h[n]
            q0 = qa[n]
            cnt[ds, q0] -= 1
            wqc[q0, c] -= 1
            room = wqc[:, c] < capw
            co = cnt[ds].astype(np.float32) - tgt[ds][:, None]
            cost = np.maximum(co + 1.0, 0.0).sum(axis=0)
            q = int(np.argmin(np.where(room, cost, np.inf)))
            qa[n] = q
            wqc[q, c] += 1
            cnt[ds, q] += 1

    maxq = cnt.max(axis=1).astype(np.int32)

    # ---- dst-side ordering: sort by maxq, deal round-robin to cores ----
    dorder = np.argsort(maxq, kind="stable")
    rank = np.empty(N, np.int64)
    rank[dorder] = np.arange(N)
    ngroups = math.ceil(npc / G)
    nslots = ngroups * G

    cnt_sorted = cnt[dorder]
    Kq = np.zeros((ngroups, NCH), np.int64)
    for g in range(ngroups):
        lo, hi = g * G * NCORES, min((g + 1) * G * NCORES, N)
        Kq[g] = cnt_sorted[lo:hi].max(axis=0)
    Kq[:, 0] = np.maximum(Kq[:, 0], 1)  # zeros-slot space for deg-0 dsts

    # ---- batches: consecutive groups, per-chunk idx count <= CAP_IDX ----
    batches = []  # (g0, B, K[4])
    g0 = 0
    while g0 < ngroups:
        B = 1
        K = Kq[g0].copy()
        while B < MAXB and g0 + B < ngroups:
            K2 = np.maximum(K, Kq[g0 + B])
            if int(((B + 1) * G * K2).max()) > CAP_IDX:
                break
            K = K2
            B += 1
        batches.append((g0, B, [int(k) for k in K]))
        g0 += B
    assert all(B * G * k <= CAP_IDX for _, B, Kb in batches for k in Kb)

    b_of_g = np.zeros(ngroups, np.int64)
    g0_of_b = np.zeros(len(batches), np.int64)
    B_of_b = np.zeros(len(batches), np.int64)
    off_qb = np.zeros((NCH, len(batches)), np.int64)  # idx offset per chunk stream
    TOTq = np.zeros(NCH, np.int64)
    for bi, (gg0, B, K) in enumerate(batches):
        b_of_g[gg0:gg0 + B] = bi
        g0_of_b[bi] = gg0
        B_of_b[bi] = B
        for q in range(NCH):
            off_qb[q, bi] = TOTq[q]
            TOTq[q] += B * G * K[q]
    assert all(t % 16 == 0 for t in TOTq)

    # ---- node table row assignment: per (owner, chunk) rank by node id ----
    keys = np.lexsort((np.arange(N), qa, owner_h))  # owner-major, chunk, id
    kq = qa[keys]
    kc = owner_h[keys]
    ng = np.r_[True, (kq[1:] != kq[:-1]) | (kc[1:] != kc[:-1])]
    st = np.flatnonzero(ng)
    seglen = np.diff(np.r_[st, N])
    rr = np.arange(N) - np.repeat(st, seglen)
    nrank = np.empty(N, np.int64)
    nrank[keys] = rr

    W_cap = int(wqc.max()) + 2
    assert NCORES * W_cap < 32768, W_cap
    node_idx = owner_h * W_cap + 2 + nrank  # row in ag chunk table

    # ---- per-edge slot positions ----
    eq = qa[src]
    okey = np.lexsort((eq, dst))
    e_d = dst[okey]
    e_q = eq[okey]
    e_i = node_idx[src[okey]].astype(np.int64)
    ngrp = np.r_[True, (e_d[1:] != e_d[:-1]) | (e_q[1:] != e_q[:-1])]
    est = np.flatnonzero(ngrp)
    e_k = np.arange(len(e_d)) - np.repeat(est, np.diff(np.r_[est, len(e_d)]))

    er = rank[e_d]
    e_core = er % NCORES
    e_p = (er // NCORES) % G
    e_g = er // (NCORES * G)
    e_bi = b_of_g[e_g]
    e_pos = (off_qb[e_q, e_bi] + e_k * (B_of_b[e_bi] * G)
             + (e_g - g0_of_b[e_bi]) * G + e_p)

    # ---- index streams (ones = pad), per core per chunk ----
    vals = [[np.ones(TOTq[q], np.int16) for q in range(NCH)]
            for _ in range(NCORES)]
    for c in range(NCORES):
        mc = e_core == c
        for q in range(NCH):
            m = mc & (e_q == q)
            vals[c][q][e_pos[m]] = e_i[m].astype(np.int16)
    # deg-0 dsts: one zeros-row slot (chunk 0, k 0) => product 0
    z = np.flatnonzero(deg == 0)
    if len(z):
        zr = rank[z]
        zc = zr % NCORES
        zp = (zr // NCORES) % G
        zg = zr // (NCORES * G)
        zbi = b_of_g[zg]
        zpos = off_qb[0, zbi] + (zg - g0_of_b[zbi]) * G + zp
        for c in range(NCORES):
            m = zc == c
            vals[c][0][zpos[m]] = 0

    # ---- wrap into idx planes [128, cols], chunks concatenated ----
    co_q = np.zeros(NCH + 1, np.int64)
    np.cumsum(TOTq // 16, out=co_q[1:])
    TOTC = int(co_q[-1])
    idx_arrs = []
    for c in range(NCORES):
        plane = np.empty((P, TOTC), np.int16)
        for q in range(NCH):
            p16 = vals[c][q].reshape(-1, 16).T  # [16, TOTq/16]
            plane[:, co_q[q]:co_q[q + 1]] = np.tile(p16, (NCORES, 1))
        idx_arrs.append(np.ascontiguousarray(plane))

    # ---- phase-1 inputs ----
    # The SPMD program is shared across cores, so the xt layout pads each
    # chunk section to the common per-chunk maximum W_sec[q]; shard row for
    # xt column j in section q is 2 + (j - secbase[q]).
    po_all = [keys[kc == c] for c in range(NCORES)]
    W_sec = [int(wqc[q].max()) for q in range(NCH)]
    NSEC = sum(W_sec)
    NPAD2 = G * math.ceil(NSEC / G)
    xt_arrs = []
    for c in range(NCORES):
        xs = np.zeros((F, NPAD2), np.float16)
        colp = 0
        for q in range(NCH):
            sel = po_all[c][qa[po_all[c]] == q]
            xs[:, colp:colp + len(sel)] = x[sel].astype(np.float16).T
            colp += W_sec[q]
        xt_arrs.append(np.ascontiguousarray(xs))
    assert all(ws + 2 <= W_cap for ws in W_sec)
    ntile = NPAD2 // G
    segs_per_tile = []
    secbase = np.zeros(NCH + 1, np.int64)
    np.cumsum(W_sec, out=secbase[1:])
    for t in range(ntile):
        lo, hi = t * G, (t + 1) * G
        segs = []
        for q in range(NCH):
            a, b = max(lo, secbase[q]), min(hi, secbase[q + 1])
            if a < b:
                segs.append((q, int(a - secbase[q]), int(a - lo), int(b - a)))
        segs_per_tile.append(segs)

    wm = np.zeros((P, 2 * R), np.float16)
    for b in range(2):
        wm[:, b * R:(b + 1) * R] = W[b * P:(b + 1) * P, :].astype(np.float16)
    import ml_dtypes
    vm = np.ascontiguousarray(V.T.astype(ml_dtypes.bfloat16))  # [64, 256]
    cz = np.zeros((2, ROWBF), ml_dtypes.bfloat16)
    cz[1] = 1.0
    im = np.ascontiguousarray(np.eye(P, dtype=ml_dtypes.bfloat16))
    dzi = np.zeros((P, 8), np.int16)  # dummy gather indices (all row 0)

    meta = dict(
        N=N, F=F, R=R, H=H, npc=npc, W_cap=W_cap,
        ngroups=ngroups, nslots=nslots, ntile=ntile, NPAD2=NPAD2,
        batches=batches, co_q=[int(v) for v in co_q],
        off_qb=off_qb.tolist(), TOTC=TOTC,
        segs_per_tile=segs_per_tile,
    )
    return meta, dorder, idx_arrs, xt_arrs, wm, vm, cz, im, dzi


def _build_program(meta):
    F = meta["F"]; R = meta["R"]; H = meta["H"]
    W_cap = meta["W_cap"]; TOTC = meta["TOTC"]
    batches = meta["batches"]; co_q = meta["co_q"]; off_qb = meta["off_qb"]
    nslots = meta["nslots"]; ntile = meta["ntile"]; NPAD2 = meta["NPAD2"]
    ngroups = meta["ngroups"]
    segs_per_tile = meta["segs_per_tile"]
    f32 = mybir.dt.float32
    f16 = mybir.dt.float16
    bf16 = mybir.dt.bfloat16
    i16 = mybir.dt.int16

    nc = bacc.Bacc(
        "TRN2", target_bir_lowering=False, debug=False,
        enable_asserts=False, num_devices=NCORES,
        num_swdge_queues=NQ,
    )
    xt = nc.dram_tensor("xt", [F, NPAD2], f16, kind="ExternalInput")
    wm = nc.dram_tensor("wm", [P, 2 * R], f16, kind="ExternalInput")
    vm = nc.dram_tensor("vm", [R, H], bf16, kind="ExternalInput")
    idx = nc.dram_tensor("idx", [P, TOTC], i16, kind="ExternalInput")
    cz = nc.dram_tensor("cz", [2, ROWBF], bf16, kind="ExternalInput")
    im = nc.dram_tensor("im", [P, P], bf16, kind="ExternalInput")
    dzi = nc.dram_tensor("dzi", [P, 8], i16, kind="ExternalInput")
    out = nc.dram_tensor("out", [nslots, H], f32, kind="ExternalOutput")

    with tile.TileContext(nc) as tc:
        with ExitStack() as ctx:
            dram = ctx.enter_context(tc.tile_pool(name="dram", bufs=1, space="DRAM"))
            sb = ctx.enter_context(tc.tile_pool(name="sb", bufs=1))

            shards = [dram.tile([W_cap, ROWBF], bf16, name=f"sh{q}")
                      for q in range(NCH)]
            ags = [dram.tile([NCORES * W_cap, ROWBF], bf16,
                             addr_space="Shared", name=f"ag{q}")
                   for q in range(NCH)]

            ixt = sb.tile([P, TOTC], i16)
            nc.sync.dma_start(out=ixt[:], in_=idx[:, :])
            w_sb = sb.tile([P, 2, R], f16)
            nc.scalar.dma_start(
                out=w_sb[:], in_=wm[:, :].rearrange("p (b r) -> p b r", b=2))
            v_sb = sb.tile([R, H], bf16)
            nc.scalar.dma_start(out=v_sb[:], in_=vm[:, :])
            im_sb = sb.tile([P, P], bf16)
            nc.scalar.dma_start(out=im_sb[:], in_=im[:, :])
            z_sb = sb.tile([2, ROWBF], bf16)
            nc.scalar.dma_start(out=z_sb[:], in_=cz[:, :])
            for q in range(NCH):
                nc.scalar.dma_start(out=shards[q][0:2, :], in_=z_sb[:])

            dz_sb = sb.tile([P, 8], i16)
            nc.sync.dma_start(out=dz_sb[:], in_=dzi[:, :])

            # ---- phase 1: h rows -> DRAM shard ----
            NXS = 4  # xt loaded in 4 slabs so matmuls start early
            tile_per_slab = math.ceil(ntile / NXS)
            with tc.tile_pool(name="xtp", bufs=1) as xtp, \
                 tc.tile_pool(name="ph1", bufs=4) as ph1, \
                 tc.tile_pool(name="ps1", bufs=4, space="PSUM") as ps1:
                xs_tiles = []
                for s in range(NXS):
                    c0 = s * tile_per_slab * G
                    c1 = min(NPAD2, (s + 1) * tile_per_slab * G)
                    if c0 >= c1:
                        break
                    xsl = xtp.tile([P, 2, c1 - c0], f16, name=f"xs{s}")
                    [nc.sync, nc.scalar][s % 2].dma_start(
                        out=xsl[:],
                        in_=xt[:, c0:c1].rearrange("(b p) n -> p b n", p=P))
                    xs_tiles.append((c0, xsl))
                for t in range(ntile):
                    c0 = t * G
                    s = min(t // tile_per_slab, len(xs_tiles) - 1)
                    sbase, xsl = xs_tiles[s]
                    ps = ps1.tile([P, R], f32, tag="ps")
                    for b in range(2):
                        nc.tensor.matmul(
                            out=ps[:],
                            lhsT=xsl[:, b, c0 - sbase:c0 - sbase + G],
                            rhs=w_sb[:, b, :],
                            start=(b == 0), stop=(b == 1),
                        )
                    hb = ph1.tile([P, R], bf16, tag="hb")
                    nc.vector.tensor_copy(out=hb[:], in_=ps[:])
                    for si, (q, r0, sb0, cnt) in enumerate(segs_per_tile[t]):
                        [nc.sync, nc.scalar][(t + si) % 2].dma_start(
                            out=shards[q][2 + r0:2 + r0 + cnt, 0:R],
                            in_=hb[sb0:sb0 + cnt, :])


            for q in range(NCH):
                nc.gpsimd.collective_compute(
                    "AllGather",
                    mybir.AluOpType.bypass,
                    replica_groups=[list(range(NCORES))],
                    ins=[shards[q][:].opt()],
                    outs=[ags[q][:].opt()],
                )
                if q == 0:
                    dm = sb.tile([P, 1, ELEM], bf16)
                    _dma_gather_raw(nc, dm[:, :, :], cz[:, 0:ELEM], dz_sb[:],
                                    P, ELEM, ROWBF, 0)

            # ---- phase 2 (batch-major): per batch, 4 chunk gathers on 4
            #      queues; folds; combine; PE transpose + V matmul ----
            with tc.tile_pool(name="stg", bufs=6) as stg, \
                 tc.tile_pool(name="ntp", bufs=3) as ntp, \
                 tc.tile_pool(name="ntb", bufs=4) as ntb, \
                 tc.tile_pool(name="ops", bufs=3) as ops, \
                 tc.tile_pool(name="pst", bufs=4, space="PSUM") as pst, \
                 tc.tile_pool(name="pso", bufs=4, space="PSUM") as pso:
                gq = 0
                # wavefront order: first 3 batches' chunks 0-1 run while
                # AG_2/AG_3 complete, then their chunks 2-3, then the rest
                WF = min(3, len(batches))
                items = []
                for bi in range(WF):
                    for q in (0, 1):
                        items.append((bi, q))
                for bi in range(WF):
                    for q in (2, 3):
                        items.append((bi, q))
                for bi in range(WF, len(batches)):
                    for q in range(NCH):
                        items.append((bi, q))
                frs_map = {bi: [] for bi in range(len(batches))}
                done_map = {bi: 0 for bi in range(len(batches))}
                npresent = [sum(1 for k in K if k > 0)
                            for (_, _, K) in batches]

                def finish(bi):
                    g0, B, K = batches[bi]
                    frs = frs_map[bi]
                    if len(frs) == 1:
                        ntf = frs[0]
                    else:
                        ntt = ntp.tile([P, MAXB, ELEM], bf16, tag="nt")
                        nc.vector.tensor_mul(
                            out=ntt[:, :B, :], in0=frs[0], in1=frs[1])
                        for fr in frs[2:]:
                            nc.vector.tensor_mul(
                                out=ntt[:, :B, :], in0=ntt[:, :B, :], in1=fr)
                        ntf = ntt[:, :B, :]
                    for j in range(B):
                        tp = pst.tile([R, G], f32, tag="tp")
                        nc.tensor.matmul(
                            out=tp[:], lhsT=ntf[:, j, :], rhs=im_sb[:],
                            start=True, stop=True)
                        nb = ntb.tile([R, G], bf16, tag="nb")
                        if j % 2 == 0:
                            nc.scalar.copy(out=nb[:], in_=tp[:])
                        else:
                            nc.vector.tensor_copy(out=nb[:], in_=tp[:])
                        po = pso.tile([P, H], f32, tag="po")
                        nc.tensor.matmul(
                            out=po[:], lhsT=nb[:], rhs=v_sb[:],
                            start=True, stop=True)
                        ob = ops.tile([P, H], f32, tag="ob")
                        if j % 2 == 0:
                            nc.vector.tensor_copy(out=ob[:], in_=po[:])
                        else:
                            nc.scalar.copy(out=ob[:], in_=po[:])
                        [nc.sync, nc.scalar][j % 2].dma_start(
                            out=out[(g0 + j) * G:(g0 + j + 1) * G, :],
                            in_=ob[:])

                for bi, q in items:
                    g0, B, K = batches[bi]
                    done_map[bi] += 1
                    if K[q] == 0:
                        if done_map[bi] == NCH:
                            finish(bi)
                        continue
                    n = B * G * K[q]
                    gt = stg.tile([P, CAPC, ELEM], bf16, tag=f"s{q}")
                    base = co_q[q] + off_qb[q][bi] // 16
                    for s0 in range(0, n, SUB):
                        ns = min(SUB, n - s0)
                        _dma_gather_raw(
                            nc, gt[:, s0 // 128:(s0 + ns) // 128, :],
                            ags[q][:, 0:ELEM],
                            ixt[:, base + s0 // 16:base + (s0 + ns) // 16],
                            ns, ELEM, ROWBF, gq % NQ)
                        gq += 1
                    if K[q] > 1:
                        v4 = gt[:, :K[q] * B, :].rearrange(
                            "p (k b) f -> p k b f", k=K[q])
                        m = K[q]
                        while m > 1:
                            if m % 2:
                                nc.vector.tensor_mul(
                                    out=v4[:, 0], in0=v4[:, 0],
                                    in1=v4[:, m - 1])
                                m -= 1
                                if m == 1:
                                    break
                            half = m // 2
                            nc.vector.tensor_mul(
                                out=v4[:, 0:half],
                                in0=v4[:, 0:half],
                                in1=v4[:, half:m])
                            m = half
                    frs_map[bi].append(gt[:, 0:B, :])
                    if done_map[bi] == NCH:
                        finish(bi)
    nc.compile()
    return nc


def kernel(x, W, V, src, dst):
    x = np.asarray(x); W = np.asarray(W); V = np.asarray(V)
    src = np.asarray(src); dst = np.asarray(dst)
    meta, dorder, idx_arrs, xt_arrs, wm, vm, cz, im, dzi = _host_prep(
        x, W, V, src, dst)
    nc = _build_program(meta)
    in_maps = [
        {"xt": xt_arrs[c],
         "wm": wm,
         "vm": vm.view(np.int16),
         "idx": idx_arrs[c],
         "cz": cz,
         "im": im.view(np.int16),
         "dzi": dzi}
        for c in range(NCORES)
    ]
    res = bass_utils.run_bass_kernel_spmd(nc, in_maps, core_ids=list(range(NCORES)))
    N, H, npc = meta["N"], meta["H"], meta["npc"]
    out_full = np.empty((N, H), dtype=np.float32)
    for c in range(NCORES):
        o = np.asarray(res.results[c]["out"]).astype(np.float32)
        nodes = dorder[np.arange(npc) * NCORES + c]
        out_full[nodes] = o[:npc]
    return out_full
